# revision 14
# baseline (speedup 1.0000x reference)
"""Deformable-attention Trainium2 kernel (8 NeuronCores, query-sharded).

Per core (2048 queries):
  q = query + query_pos; qT via PE transpose.
  v = value @ W_val + b_val (replicated), staged per y-quarter.
  Patch table T[l,h][x*128+y, 128] bf16 in HBM: entry = 2x2xDH corner
  block (yy, xx, dh) = 256B.
  off/attn projections; softmax over (l,p) groups of 16 (in-place).
  Branchless bilinear weights * attn -> w4[qp, c, X], X=(l,h,p,qhh)=2048,
  computed in 256-column blocks.
  idx = s*128+t int16 wrapped to dma_gather layout via identity-slice
  PE matmuls (wrapped col = 8*X + qlo).
  16 chunks x 4 sub-gathers of 4096: dma_gather 256B elems ->
  Gt[q%128, stripe, (c,dh)]; M = Gt*w4; corner/point/level sums -> OH.
  out = OHT.T @ W_out + b_out + query.
"""
import numpy as np

P = 128
NQ_FULL = 16384
NQC = 2048
C = 256
HEADS = 8
POINTS = 8
LEVELS = 2
DH = 32
QH = 16
NCORES = 8
X = LEVELS * HEADS * POINTS * QH
NCHUNK = LEVELS * HEADS
SUBS = 4
SUBIDX = 4096
BLK = 256            # weight-phase column block (2 heads' worth)
NBLK = X // BLK
DEBUG = False
TRUNC_CAST = False    # f32->i32 DVE cast truncates; False -> round-nearest

_CACHE = {}


def _build():
    import concourse.bacc as bacc
    import concourse.mybir as mybir
    from concourse.tile import TileContext
    from concourse import library_config
    from concourse.masks import make_identity
    from contextlib import ExitStack

    fp32 = mybir.dt.float32
    bf16 = mybir.dt.bfloat16
    i32 = mybir.dt.int32
    i16 = mybir.dt.int16
    OP = mybir.AluOpType
    AF = mybir.ActivationFunctionType

    nc = bacc.Bacc("TRN2")

    d_query = nc.dram_tensor("query", [NQC, C], fp32, kind="ExternalInput")
    d_qpos = nc.dram_tensor("query_pos", [NQC, C], fp32, kind="ExternalInput")
    d_value = nc.dram_tensor("value", [LEVELS, NQ_FULL, C], fp32, kind="ExternalInput")
    d_ref = nc.dram_tensor("refp", [NQC, LEVELS, 2], fp32, kind="ExternalInput")
    d_Woff = nc.dram_tensor("W_off", [C, C], fp32, kind="ExternalInput")
    d_boff = nc.dram_tensor("b_off", [1, C], fp32, kind="ExternalInput")
    d_Watt = nc.dram_tensor("W_attn", [C, P], fp32, kind="ExternalInput")
    d_batt = nc.dram_tensor("b_attn", [1, P], fp32, kind="ExternalInput")
    d_Wval = nc.dram_tensor("W_val", [C, C], fp32, kind="ExternalInput")
    d_bval = nc.dram_tensor("b_val", [1, C], fp32, kind="ExternalInput")
    d_Wout = nc.dram_tensor("W_out", [C, C], fp32, kind="ExternalInput")
    d_bout = nc.dram_tensor("b_out", [1, C], fp32, kind="ExternalInput")
    d_out = nc.dram_tensor("out", [NQC, C], fp32, kind="ExternalOutput")
    dbg = {}
    if DEBUG:
        dbg["w4"] = nc.dram_tensor("dbg_w4", [P, 4 * X], bf16, kind="ExternalOutput")
        dbg["idxw"] = nc.dram_tensor("dbg_idxw", [P, NCHUNK * 1024], i16,
                                     kind="ExternalOutput")
        dbg["gt"] = nc.dram_tensor("dbg_gt", [P, SUBS * 4096], bf16,
                                   kind="ExternalOutput")
        dbg["oh"] = nc.dram_tensor("dbg_oh", [P, QH * C], bf16, kind="ExternalOutput")

    with TileContext(nc) as tc, ExitStack() as ctx:
        nc.gpsimd.load_library(library_config.mlp)

        cpool = ctx.enter_context(tc.tile_pool(name="consts", bufs=1))
        spool = ctx.enter_context(tc.tile_pool(name="work", bufs=2))
        vpool = ctx.enter_context(tc.tile_pool(name="vsb", bufs=1))
        tchp = ctx.enter_context(tc.tile_pool(name="tch", bufs=2))
        wpool = ctx.enter_context(tc.tile_pool(name="wts", bufs=1))
        bpool = ctx.enter_context(tc.tile_pool(name="wblk", bufs=1))
        gpool = ctx.enter_context(tc.tile_pool(name="gath", bufs=2))
        mpool = ctx.enter_context(tc.tile_pool(name="mul", bufs=2))
        rpool = ctx.enter_context(tc.tile_pool(name="red", bufs=2))
        psum = ctx.enter_context(tc.tile_pool(name="ps", bufs=1, space="PSUM"))
        dpool = ctx.enter_context(tc.tile_pool(name="tdram", bufs=NCHUNK,
                                               space="DRAM"))

        # -------- constants --------
        ident_b = cpool.tile([P, P], bf16, tag="idb")
        make_identity(nc, ident_b)
        ident_f = cpool.tile([P, P], fp32, tag="idf")
        make_identity(nc, ident_f)
        shift_b = cpool.tile([P, P], bf16, tag="shb")   # shift[k,m]=I[k,m+1 mod P]
        nc.vector.tensor_copy(out=shift_b[:, 0:P - 1], in_=ident_b[:, 1:P])
        nc.vector.tensor_copy(out=shift_b[:, P - 1:P], in_=ident_b[:, 0:1])

        def bias_bcast(dram, n):
            t1 = cpool.tile([1, n], fp32, tag=f"b1_{dram.name}")
            nc.sync.dma_start(out=t1, in_=dram[:])
            tb = cpool.tile([P, n], fp32, tag=f"bb_{dram.name}")
            nc.gpsimd.partition_broadcast(tb, t1)
            return tb

        boff_b = bias_bcast(d_boff, C)
        batt_b = bias_bcast(d_batt, P)
        bval_b = bias_bcast(d_bval, C)
        bout_b = bias_bcast(d_bout, C)

        def wload(dram, cols):
            t = cpool.tile([P, 2, cols], bf16, tag=f"w_{dram.name}")
            for half in range(2):
                nc.gpsimd.dma_start(out=t[:, half, :],
                                    in_=dram[half * P:(half + 1) * P, :])
            return t

        Woff_b = wload(d_Woff, C)
        Watt_b = wload(d_Watt, P)
        Wval_b = wload(d_Wval, C)
        Wout_b = wload(d_Wout, C)

        # -------- q prep --------
        qT = cpool.tile([P, 2, NQC], bf16, tag="qT")
        for t in range(QH):
            qa = spool.tile([P, C], bf16, tag="qa")
            qb = spool.tile([P, C], bf16, tag="qb")
            nc.gpsimd.dma_start(out=qa, in_=d_query[t * P:(t + 1) * P, :])
            nc.gpsimd.dma_start(out=qb, in_=d_qpos[t * P:(t + 1) * P, :])
            nc.vector.tensor_add(out=qa, in0=qa, in1=qb)
            for half in range(2):
                pt = psum.tile([P, P], bf16, tag="ptr", bufs=2)
                nc.tensor.transpose(out=pt, in_=qa[:, half * P:(half + 1) * P],
                                    identity=ident_b)
                nc.vector.tensor_copy(out=qT[:, half, t * P:(t + 1) * P], in_=pt)

        # -------- off/attn projections --------
        off_sb = wpool.tile([P, QH, C], bf16, tag="off")
        att_sb = wpool.tile([P, QH, P], fp32, tag="attl")
        for t in range(QH):
            po = psum.tile([P, C], fp32, tag="po", bufs=1)
            for half in range(2):
                nc.tensor.matmul(out=po, lhsT=qT[:, half, t * P:(t + 1) * P],
                                 rhs=Woff_b[:, half, :], start=half == 0,
                                 stop=half == 1)
            nc.vector.tensor_add(out=off_sb[:, t, :], in0=po, in1=boff_b)
            pa = psum.tile([P, P], fp32, tag="po", bufs=1)
            for half in range(2):
                nc.tensor.matmul(out=pa, lhsT=qT[:, half, t * P:(t + 1) * P],
                                 rhs=Watt_b[:, half, :], start=half == 0,
                                 stop=half == 1)
            nc.vector.tensor_add(out=att_sb[:, t, :], in0=pa, in1=batt_b)

        # -------- softmax over innermost 16 (in place) --------
        att3 = att_sb.rearrange("p q c -> p (q c)").rearrange(
            "p (g s) -> p g s", s=16)
        mx = wpool.tile([P, P], fp32, tag="mx")
        nc.vector.tensor_reduce(out=mx, in_=att3, axis=mybir.AxisListType.X,
                                op=OP.max)
        nc.vector.tensor_tensor(out=att3, in0=att3,
                                in1=mx.unsqueeze(2).broadcast_to([P, P, 16]),
                                op=OP.subtract)
        nc.scalar.activation(out=att3, in_=att3, func=AF.Exp)
        sm = wpool.tile([P, P], fp32, tag="sm")
        nc.vector.tensor_reduce(out=sm, in_=att3, axis=mybir.AxisListType.X,
                                op=OP.add)
        nc.vector.reciprocal(out=sm, in_=sm)
        nc.vector.tensor_tensor(out=att3, in0=att3,
                                in1=sm.unsqueeze(2).broadcast_to([P, P, 16]),
                                op=OP.mult)

        # -------- ref points --------
        ref_sb = wpool.tile([P, QH, LEVELS, 2], fp32, tag="ref")
        nc.gpsimd.dma_start(
            out=ref_sb,
            in_=d_ref[:].rearrange("(qh qp) l x -> qp qh l x", qp=P))
        nc.vector.tensor_scalar(out=ref_sb, in0=ref_sb, scalar1=128.0,
                                scalar2=3.5, op0=OP.mult, op1=OP.add)

        # -------- bilinear weights + idx, blocked over X --------
        # views in (l, h, pt, q) order
        off_v = [off_sb.rearrange("p q (h l pt xy) -> p xy l h pt q",
                                  h=HEADS, l=LEVELS, pt=POINTS, xy=2)[:, xy]
                 for xy in (0, 1)]
        ref_v = [ref_sb.rearrange("p q l x -> p x l q")[:, xy]
                 .unsqueeze(2).unsqueeze(3)
                 .broadcast_to([P, LEVELS, HEADS, POINTS, QH])
                 for xy in (0, 1)]
        attn_v = att_sb.rearrange("p q (h l pt) -> p l h pt q",
                                  h=HEADS, l=LEVELS, pt=POINTS)

        w4 = wpool.tile([P, 4, X], bf16, tag="w4")
        idxw = wpool.tile([P, NCHUNK * 1024], i16, tag="idxw")
        idxw8 = idxw[0:16, :].rearrange("p (c e) -> p c e", e=8)

        HB = BLK // (POINTS * QH)  # heads per block = 2
        for b in range(NBLK):
            l_b, h0 = divmod(b * HB, HEADS)
            ab = {}
            for xy in (0, 1):
                px4 = bpool.tile([P, BLK], fp32, tag="px4")
                pxv = px4.rearrange("p (h pt q) -> p h pt q", h=HB, pt=POINTS)
                nc.vector.tensor_tensor(out=pxv,
                                        in0=off_v[xy][:, l_b, h0:h0 + HB],
                                        in1=ref_v[xy][:, l_b, h0:h0 + HB],
                                        op=OP.add)
                if not TRUNC_CAST:
                    nc.vector.tensor_scalar(out=px4, in0=px4, scalar1=-0.5,
                                            scalar2=None, op0=OP.add)
                f4i = bpool.tile([P, BLK], i32, tag="f4i")
                nc.vector.tensor_copy(out=f4i, in_=px4)
                f4 = bpool.tile([P, BLK], fp32, tag="f4")
                nc.vector.tensor_copy(out=f4, in_=f4i)
                if not TRUNC_CAST:
                    nc.vector.tensor_scalar(out=px4, in0=px4, scalar1=0.5,
                                            scalar2=None, op0=OP.add)
                w1 = bpool.tile([P, BLK], fp32, tag="w1")
                nc.vector.tensor_tensor(out=w1, in0=px4, in1=f4, op=OP.subtract)
                s4 = bpool.tile([P, BLK], fp32, tag=f"s4_{xy}")
                nc.vector.tensor_scalar(out=s4, in0=f4, scalar1=4.0,
                                        scalar2=130.0, op0=OP.max, op1=OP.min)
                f4p = bpool.tile([P, BLK], fp32, tag="f4p")
                nc.vector.tensor_scalar(out=f4p, in0=f4, scalar1=1.0,
                                        scalar2=None, op0=OP.add)
                s4p = bpool.tile([P, BLK], fp32, tag="f4p")
                nc.vector.tensor_scalar(out=s4p, in0=s4, scalar1=1.0,
                                        scalar2=None, op0=OP.add)
                e0 = bpool.tile([P, BLK], fp32, tag="e0")
                nc.vector.tensor_tensor(out=e0, in0=s4, in1=f4, op=OP.is_equal)
                e1 = bpool.tile([P, BLK], fp32, tag="e1")
                nc.vector.tensor_tensor(out=e1, in0=s4, in1=f4p, op=OP.is_equal)
                e2 = bpool.tile([P, BLK], fp32, tag="e2")
                nc.vector.tensor_tensor(out=e2, in0=s4p, in1=f4, op=OP.is_equal)
                d0 = bpool.tile([P, BLK], fp32, tag="d0")
                nc.vector.tensor_tensor(out=d0, in0=e1, in1=e0, op=OP.subtract)
                nc.vector.tensor_tensor(out=d0, in0=d0, in1=w1, op=OP.mult)
                a0 = bpool.tile([P, BLK], fp32, tag=f"a0_{xy}")
                nc.vector.tensor_tensor(out=a0, in0=e0, in1=d0, op=OP.add)
                d1 = bpool.tile([P, BLK], fp32, tag="d0")
                nc.vector.tensor_tensor(out=d1, in0=e0, in1=e2, op=OP.subtract)
                nc.vector.tensor_tensor(out=d1, in0=d1, in1=w1, op=OP.mult)
                a1 = bpool.tile([P, BLK], fp32, tag=f"a1_{xy}")
                nc.vector.tensor_tensor(out=a1, in0=e2, in1=d1, op=OP.add)
                if xy == 1:
                    for a in (a0, a1):
                        av = a.rearrange("p (h pt q) -> p h pt q", h=HB,
                                         pt=POINTS)
                        nc.vector.tensor_tensor(
                            out=av, in0=av,
                            in1=attn_v[:, l_b, h0:h0 + HB], op=OP.mult)
                ab[f"a0_{xy}"] = a0
                ab[f"a1_{xy}"] = a1
                ab[f"s4_{xy}"] = s4
            bs = slice(b * BLK, (b + 1) * BLK)
            for yy in (0, 1):
                for xx in (0, 1):
                    nc.vector.tensor_tensor(out=w4[:, yy * 2 + xx, bs],
                                            in0=ab[f"a{yy}_1"],
                                            in1=ab[f"a{xx}_0"], op=OP.mult)
            idxf = bpool.tile([P, BLK], fp32, tag="idxf")
            nc.vector.tensor_scalar(out=idxf, in0=ab["s4_0"], scalar1=128.0,
                                    scalar2=-516.0, op0=OP.mult, op1=OP.add)
            nc.vector.tensor_tensor(out=idxf, in0=idxf, in1=ab["s4_1"],
                                    op=OP.add)
            for qlo in range(8):
                pf = psum.tile([16, BLK], fp32, tag="pfold", bufs=1)
                nc.tensor.matmul(out=pf,
                                 lhsT=ident_f[:, qlo * 16:qlo * 16 + 16],
                                 rhs=idxf, start=True, stop=True)
                nc.vector.tensor_copy(out=idxw8[:, bs, qlo], in_=pf)
        for k in range(1, 8):
            nc.sync.dma_start(out=idxw[k * 16:(k + 1) * 16, :], in_=idxw[0:16, :])
        if DEBUG:
            nc.sync.dma_start(out=dbg["idxw"][:], in_=idxw)
            nc.sync.dma_start(out=dbg["w4"][:],
                              in_=w4.rearrange("p c x -> p (c x)"))

        # -------- value proj + patch tables (per y-quarter) --------
        T_tiles = [dpool.tile([NQ_FULL, P], bf16, tag=f"T{i}", name=f"Ttab{i}")
                   for i in range(NCHUNK)]
        for l in range(LEVELS):
            for yq in range(4):
                y0 = yq * 32
                nrow = 33 if yq < 3 else 32
                v_q = vpool.tile([P, 33, C], bf16, tag="vq")
                for yi in range(nrow):
                    y = y0 + yi
                    vb = spool.tile([P, C], bf16, tag="vb")
                    nc.gpsimd.dma_start(out=vb,
                                        in_=d_value[l, y * P:(y + 1) * P, :])
                    vT = spool.tile([P, 2, P], bf16, tag="vT")
                    for half in range(2):
                        ptx = psum.tile([P, P], bf16, tag="ptr", bufs=2)
                        nc.tensor.transpose(out=ptx,
                                            in_=vb[:, half * P:(half + 1) * P],
                                            identity=ident_b)
                        nc.vector.tensor_copy(out=vT[:, half, :], in_=ptx)
                    pv = psum.tile([P, C], fp32, tag="pv", bufs=1)
                    for half in range(2):
                        nc.tensor.matmul(out=pv, lhsT=vT[:, half, :],
                                         rhs=Wval_b[:, half, :],
                                         start=half == 0, stop=half == 1)
                    nc.vector.tensor_add(out=v_q[:, yi, :], in0=pv, in1=bval_b)
                if yq == 3:
                    nc.vector.memset(v_q[:, 32, :], 0)
                for h in range(HEADS):
                    # x-shifted rows via PE: pvs[x, :] = v_q[x+1, h-slice]
                    pvs = psum.tile([P, 33, DH], fp32, tag="pvs", bufs=1)
                    vqh = v_q[:, :, h * DH:(h + 1) * DH]
                    for a0 in (0, 16, 32):
                        na = min(16, 33 - a0)
                        nc.tensor.matmul(out=pvs[:, a0:a0 + na, :],
                                         lhsT=shift_b,
                                         rhs=vqh[:, a0:a0 + na, :],
                                         start=True, stop=True)
                    tch = tchp.tile([P, 32, 4, DH], bf16, tag="tch")
                    for yy in (0, 1):
                        nc.vector.tensor_copy(
                            out=tch[:, :, yy * 2, :],
                            in_=v_q[:, yy:yy + 32, h * DH:(h + 1) * DH])
                        nc.scalar.activation(
                            out=tch[:, :, yy * 2 + 1, :],
                            in_=pvs[:, yy:yy + 32, :], func=AF.Copy)
                    nc.sync.dma_start(
                        out=T_tiles[l * HEADS + h][:].rearrange(
                            "(x y) c -> x y c", x=P)[:, y0:y0 + 32, :],
                        in_=tch)

        # -------- gather + weighted reduce --------
        OH = rpool.tile([P, QH, C], bf16, tag="OH", bufs=1)
        acc0 = {}
        for ch in range(NCHUNK):
            l, h = divmod(ch, HEADS)
            if l == 0:
                acc = rpool.tile([P, QH, DH], bf16, tag=f"acc0_{h}", bufs=1)
                acc0[h] = acc
            else:
                acc = rpool.tile([P, QH, DH], bf16, tag="acc1", bufs=2)
            for sub in range(SUBS):
                gt = gpool.tile([P, 32, 4, DH], bf16, tag="gt")
                gt3 = gt.rearrange("p a c d -> p (a c d)").rearrange(
                    "p (s e) -> p s e", e=P)
                for s4 in range(4):
                    c0 = ch * 1024 + sub * 256 + s4 * 64
                    nc.gpsimd.dma_gather(
                        gt3[:, s4 * 8:(s4 + 1) * 8, :],
                        T_tiles[ch][:],
                        idxw[:, c0:c0 + 64],
                        1024, 1024, P, elem_step=P)
                if DEBUG and ch == 0:
                    nc.sync.dma_start(
                        out=dbg["gt"][:, sub * 4096:(sub + 1) * 4096],
                        in_=gt.rearrange("p a c d -> p (a c d)"))
                m = mpool.tile([P, 32, 4, DH], bf16, tag="m")
                w4ap = w4[:, :, ch * P + sub * 32: ch * P + (sub + 1) * 32] \
                    .rearrange("p c s -> p s c").unsqueeze(3) \
                    .broadcast_to([P, 32, 4, DH])
                nc.vector.tensor_tensor(out=m, in0=gt, in1=w4ap, op=OP.mult)
                r01 = mpool.tile([P, 32, DH], bf16, tag="r01", bufs=1)
                nc.vector.tensor_tensor(out=r01, in0=m[:, :, 0, :],
                                        in1=m[:, :, 1, :], op=OP.add)
                r23 = mpool.tile([P, 32, DH], bf16, tag="r23", bufs=1)
                nc.vector.tensor_tensor(out=r23, in0=m[:, :, 2, :],
                                        in1=m[:, :, 3, :], op=OP.add)
                radd = mpool.tile([P, 2, QH, DH], bf16, tag="radd", bufs=1)
                nc.vector.tensor_tensor(out=radd, in0=r01, in1=r23, op=OP.add)
                if sub == 0:
                    nc.vector.tensor_tensor(out=acc, in0=radd[:, 0],
                                            in1=radd[:, 1], op=OP.add)
                else:
                    ph = mpool.tile([P, QH, DH], bf16, tag="ph", bufs=1)
                    nc.vector.tensor_tensor(out=ph, in0=radd[:, 0],
                                            in1=radd[:, 1], op=OP.add)
                    nc.vector.tensor_tensor(out=acc, in0=acc, in1=ph, op=OP.add)
            if l == 1:
                nc.vector.tensor_tensor(out=OH[:, :, h * DH:(h + 1) * DH],
                                        in0=acc0[h], in1=acc, op=OP.add)
        if DEBUG:
            nc.sync.dma_start(out=dbg["oh"][:],
                              in_=OH.rearrange("p q c -> p (q c)"))

        # -------- output projection --------
        OHT = rpool.tile([P, 2, NQC], bf16, tag="OHT", bufs=1)
        for t in range(QH):
            for half in range(2):
                pt = psum.tile([P, P], bf16, tag="ptr", bufs=2)
                nc.tensor.transpose(out=pt,
                                    in_=OH[:, t, half * P:(half + 1) * P],
                                    identity=ident_b)
                nc.vector.tensor_copy(out=OHT[:, half, t * P:(t + 1) * P],
                                      in_=pt)
        for t in range(QH):
            pout = psum.tile([P, C], fp32, tag="po", bufs=1)
            for half in range(2):
                nc.tensor.matmul(out=pout, lhsT=OHT[:, half, t * P:(t + 1) * P],
                                 rhs=Wout_b[:, half, :],
                                 start=half == 0, stop=half == 1)
            qf = spool.tile([P, C], fp32, tag="qf")
            nc.sync.dma_start(out=qf, in_=d_query[t * P:(t + 1) * P, :])
            osb = spool.tile([P, C], fp32, tag="osb")
            nc.vector.tensor_add(out=osb, in0=pout, in1=bout_b)
            nc.vector.tensor_add(out=osb, in0=osb, in1=qf)
            nc.sync.dma_start(out=d_out[t * P:(t + 1) * P, :], in_=osb)

    nc.compile()
    return nc


def kernel(query, query_pos, value, reference_points, spatial_shapes,
           W_off, b_off, W_attn, b_attn, W_val, b_val, W_out, b_out):
    import sys
    if "/opt/trn_rl_repo" not in sys.path:
        sys.path.insert(0, "/opt/trn_rl_repo")
    from concourse.bass_utils import run_bass_kernel_spmd

    if "nc" not in _CACHE:
        _CACHE["nc"] = _build()
    nc = _CACHE["nc"]

    f = np.float32
    com = {
        "value": np.ascontiguousarray(value, f),
        "W_off": np.ascontiguousarray(W_off, f),
        "b_off": np.ascontiguousarray(b_off, f).reshape(1, C),
        "W_attn": np.ascontiguousarray(W_attn, f),
        "b_attn": np.ascontiguousarray(b_attn, f).reshape(1, P),
        "W_val": np.ascontiguousarray(W_val, f),
        "b_val": np.ascontiguousarray(b_val, f).reshape(1, C),
        "W_out": np.ascontiguousarray(W_out, f),
        "b_out": np.ascontiguousarray(b_out, f).reshape(1, C),
    }
    in_maps = []
    for c in range(NCORES):
        sl = slice(c * NQC, (c + 1) * NQC)
        in_maps.append(dict(
            com,
            query=np.ascontiguousarray(query[0, sl], f),
            query_pos=np.ascontiguousarray(query_pos[0, sl], f),
            refp=np.ascontiguousarray(reference_points[0, sl], f),
        ))
    res = run_bass_kernel_spmd(nc, in_maps, core_ids=list(range(NCORES)),
                               **_CACHE.get("run_kwargs", {}))
    _CACHE["last_result"] = res
    out = np.concatenate([res.results[c]["out"] for c in range(NCORES)], axis=0)
    return out[None]
